# revision 11
# baseline (speedup 1.0000x reference)
"""Trainium2 Bass kernel for nn_CrossmotionModule (gnn_message_passing).

Reference computation (B=4, M=256, T=64, Dm=512, E=768):
    rel[b,m,t,n,k] = (c[b,m,t,k] - c[b,n,t,k]) * vis[b,m,t] * vis[b,n,t]
    fea[b,t,m,(n,k)] = rel                  # (B,T,M,512)
    h   = fea @ W1 + b1                     # (B,T,M,512)
    out = [h, pos] @ W2 + b2                # (B,T,M,768)

Algebraic collapse: with p = vis (B,T,M), u0 = p*c0, u1 = p*c1, the output is
a rank-3 outer product per (b,t) plus a constant:
    out[bt,m,e] = u0[m]*G0[e] + u1[m]*G1[e] - p[m]*G2[e] + const[m,e]
where, with the host-folded fused weight V2 = W1 @ W2[:512] (512, 768):
    G0[e] = sum_n p[n] V2[2n, e]
    G1[e] = sum_n p[n] V2[2n+1, e]
    G2[e] = sum_nk (p*c)[nk] V2[nk, e]
    const = b1 @ W2[:512] + pos @ W2[512:] + b2

Single-pass bf16 pipeline (tolerance is rel_l2 < 2e-2; this lands ~3e-3):
bf16 in / f32 PSUM accumulate / bf16 out, host upcasts to float32. When
const is m-independent (pos_embed == 0, always true here) it is folded into
the matmul as a 4th contraction row (lhsT ones, rhs const_e), so the PSUM
drain is a pure cast-copy, split per 512-col PSUM bank between the DVE and
ACT engines. Input/bounce DMAs dispatch from gpsimd (cheap parallel queue);
output DMAs alternate sync/gpsimd.

Sharding: data-parallel over bt = (b,t) flattened; 256 rows / 8 cores = 32
rows per core. Weights replicated. No cross-device communication.
"""

import ml_dtypes
import numpy as np

B, M, T = 4, 256, 64
D_MOT, D_ABS, D_OUT = 512, 512, 768
N_CORES = 8
BT = B * T            # 256
R = BT // N_CORES     # 32 bt rows per core
E = D_OUT

BF16 = ml_dtypes.bfloat16

_CACHED_NC = {}


def _build_nc(fold_const):
    """Build the SPMD Bass program (identical for all 8 cores)."""
    import concourse.bacc as bacc
    import concourse.bass as bass
    import concourse.mybir as mybir
    import concourse.tile as tile

    f32 = mybir.dt.float32
    bf16 = mybir.dt.bfloat16
    PSUM = bass.MemorySpace.PSUM

    nc = bacc.Bacc("TRN2", target_bir_lowering=False, debug=False)

    # Per-core inputs (host-prepared layouts; see _prep_inputs).
    # lv: per k-chunk [la_k (96) | vh_k (768)] so one DMA unblocks matmul k.
    lv_d = nc.dram_tensor("lv", [128, 4 * 864], bf16, kind="ExternalInput")
    ut4_d = nc.dram_tensor("ut4", [4, R * 256], bf16, kind="ExternalInput")
    if fold_const:
        cst_d = nc.dram_tensor("cst", [1, R * E], bf16, kind="ExternalInput")
    else:
        cst_d = nc.dram_tensor("cst", [128, 1536], f32, kind="ExternalInput")
    out_d = nc.dram_tensor("out", [R, M, E], bf16, kind="ExternalOutput")
    # DRAM bounce for the G reshuffle. Row (ck*3 + j) holds chunk ck of Gj:
    # 8 consecutive r values x 768 e. Written as a (j r8) partition scatter
    # per chunk, read back as 3 fat rows per chunk.
    gscr_d = nc.dram_tensor("gscr", [12, 8 * E], bf16)

    CK = (R // 4) * E  # per-chunk column count of g4's (r e) axis

    with tile.TileContext(nc) as tc:
        with tc.tile_pool(name="persist", bufs=1) as pers:
            ut4_sb = pers.tile([4, R * 256], bf16)
            g4_sb = pers.tile([4, R * E], bf16)
            scr_sb = pers.tile([1, 8], f32)
            if not fold_const:
                cst_sb = pers.tile([128, 1536], f32)

            # ---- prologue: G[(ck,j,r8), e] via the fused weight V2 ----
            with (
                tc.tile_pool(name="pro", bufs=1) as pro,
                tc.tile_pool(name="prop", bufs=1, space=PSUM) as prop,
            ):
                lv_sb = pro.tile([128, 4 * 864], bf16)
                g96_sb = pro.tile([96, E], bf16)
                # Spread input dispatches across queues so no single
                # sequencer serializes the prologue (HWDGE = SP + ACT only).
                for kc in range(4):
                    eng = nc.scalar if kc < 2 else nc.sync
                    eng.dma_start(
                        lv_sb[:, kc * 864 : (kc + 1) * 864],
                        lv_d[:, kc * 864 : (kc + 1) * 864],
                    )
                nc.gpsimd.dma_start(ut4_sb[:], ut4_d[:])
                if fold_const:
                    for ck in range(4):
                        nc.gpsimd.dma_start(
                            g4_sb[3:4, ck * CK : (ck + 1) * CK],
                            cst_d[:, ck * CK : (ck + 1) * CK],
                        )
                else:
                    nc.gpsimd.dma_start(cst_sb[:], cst_d[:])
                # Warm the ACT activation table (after ACT's dma dispatches).
                nc.gpsimd.memset(scr_sb[:], 0.0)
                nc.scalar.copy(scr_sb[0:1, 4:8], scr_sb[0:1, 0:4])

                g_ps = prop.tile([96, E], f32)
                for kc in range(4):
                    for lo, hi in ((0, 512), (512, 768)):
                        nc.tensor.matmul(
                            g_ps[:, lo:hi],
                            lv_sb[:, kc * 864 : kc * 864 + 96],
                            lv_sb[:, kc * 864 + 96 + lo : kc * 864 + 96 + hi],
                            start=(kc == 0),
                            stop=(kc == 3),
                        )
                # Cast all of G (DVE partition starts must be 32-aligned),
                # then per chunk: scatter-write, fat read-back.
                nc.vector.tensor_copy(g96_sb[:], g_ps[:])
                for ck in range(4):
                    p0 = ck * 24
                    nc.gpsimd.dma_start(
                        gscr_d[3 * ck : 3 * ck + 3].rearrange(
                            "j (r e) -> (j r) e", r=8
                        ),
                        g96_sb[p0 : p0 + 24, :],
                    )
                    nc.sync.dma_start(
                        g4_sb[0:3, ck * CK : (ck + 1) * CK],
                        gscr_d[3 * ck : 3 * ck + 3],
                    )

            # ---- main loop: out[r, m, e] = U4_r^T G4_r (+ const) ----
            # m = 2p + w: partition p holds (w=0, w=1) halves side by side so
            # each DMA line is 3KB contiguous in DRAM. PSUM tiles are single
            # banks (512 f32) so drains hand banks back at fine grain.
            with (
                tc.tile_pool(name="mp", bufs=8, space=PSUM) as mp,
                tc.tile_pool(name="op", bufs=8) as op,
            ):
                for r in range(R):
                    u_w0 = ut4_sb[:, r * 256 : r * 256 + 128]
                    u_w1 = ut4_sb[:, r * 256 + 128 : r * 256 + 256]
                    g0 = r * E
                    ps0 = mp.tile([128, 512], f32, tag="ps")
                    ps1 = mp.tile([128, 512], f32, tag="ps")
                    ps2 = mp.tile([128, 512], f32, tag="ps")
                    nc.tensor.matmul(ps0[:], u_w0, g4_sb[:, g0 : g0 + 512])
                    nc.tensor.matmul(ps1[:, 0:256], u_w0, g4_sb[:, g0 + 512 : g0 + 768])
                    nc.tensor.matmul(ps1[:, 256:512], u_w1, g4_sb[:, g0 : g0 + 256])
                    nc.tensor.matmul(ps2[:], u_w1, g4_sb[:, g0 + 256 : g0 + 768])
                    out_sb = op.tile([128, 1536], bf16, tag="out_sb")
                    if fold_const:
                        # Alternate engines across banks and rows for balance.
                        drains = (
                            (nc.vector.tensor_copy, nc.scalar.copy, nc.vector.tensor_copy)
                            if r % 2 == 0
                            else (nc.scalar.copy, nc.vector.tensor_copy, nc.scalar.copy)
                        )
                        drains[0](out_sb[:, 0:512], ps0[:])
                        drains[1](out_sb[:, 512:1024], ps1[:])
                        drains[2](out_sb[:, 1024:1536], ps2[:])
                    else:
                        nc.vector.tensor_add(out_sb[:, 0:512], ps0[:], cst_sb[:, 0:512])
                        nc.vector.tensor_add(out_sb[:, 512:1024], ps1[:], cst_sb[:, 512:1024])
                        nc.vector.tensor_add(out_sb[:, 1024:1536], ps2[:], cst_sb[:, 1024:1536])
                    dst = out_d[r].rearrange("(p w) e -> p (w e)", w=2)
                    if r == R - 1:
                        # Split the last row so the tail transfer is short.
                        nc.sync.dma_start(dst[:, 0:768], out_sb[:, 0:768])
                        nc.sync.dma_start(dst[:, 768:1536], out_sb[:, 768:1536])
                    else:
                        nc.sync.dma_start(dst, out_sb[:])
    nc.compile()
    return nc


def _prep_inputs(coords, mask, pos, w1, b1, w2, b2):
    """Host-side input sharding + weight-only constant folding."""
    nan0 = np.isnan(coords[..., 0])
    c = np.nan_to_num(coords)
    vis = np.where(nan0, np.float32(0.0), mask).astype(np.float32)

    p_all = np.ascontiguousarray(vis.transpose(0, 2, 1)).reshape(BT, M)
    c_bt = np.ascontiguousarray(c.transpose(0, 2, 1, 3)).reshape(BT, M, 2)
    q_all = (p_all[:, :, None] * c_bt).reshape(BT, 2 * M).astype(np.float32)

    W2t = w2[:D_MOT]
    W2b = w2[D_MOT:]
    const = (b1 @ W2t + b2)[None, :] + pos @ W2b          # (M, 768)
    fold_const = bool(np.all(const == const[0:1]))
    if fold_const:
        cst_dev = np.tile(const[0].astype(BF16)[None, :], (1, R)).reshape(1, R * E)
    else:
        cst_dev = np.ascontiguousarray(
            const.astype(np.float32).reshape(128, 2, D_OUT)
        ).reshape(128, 1536)

    # Fused weight V2 = W1 @ W2t in bf16, 128 contraction rows per chunk.
    v2h = (w1 @ W2t).astype(np.float32).astype(BF16)      # (512, 768)
    v2h = v2h.reshape(4, 128, D_OUT)                      # (kc, kp, e)

    # U4 rows pair with G4 rows [G0; G1; G2; const]: [u0; u1; -p; ones].
    u0 = q_all[:, 0::2]
    u1 = q_all[:, 1::2]
    ones = np.ones_like(p_all)
    U4 = np.stack([u0, u1, -p_all, ones], axis=0)         # (4, BT, M)
    U4 = U4.reshape(4, BT, 128, 2).transpose(0, 1, 3, 2)  # m = 2p+w
    U4 = U4.astype(BF16)                                  # (4, BT, 2, 128)

    in_maps = []
    for i in range(N_CORES):
        rows = slice(i * R, (i + 1) * R)
        ut4_i = np.ascontiguousarray(U4[:, rows]).reshape(4, R * 256)

        # L columns ordered (ck, j, r8) so bounce chunks are partition-
        # contiguous: col ck*24 + j*8 + r8 <-> G row j of bt row ck*8+r8.
        # L rows (the 512 contraction dim): j=0 picks even k (G0), j=1 odd
        # (G1), j=2 is Q (G2).
        pc_t = p_all[rows].T                              # (256, R)
        qc_t = q_all[rows].T                              # (512, R)
        la = np.zeros((512, 96), np.float32)
        for ck in range(4):
            rs = slice(ck * 8, ck * 8 + 8)
            cb = ck * 24
            la[0::2, cb : cb + 8] = pc_t[:, rs]
            la[1::2, cb + 8 : cb + 16] = pc_t[:, rs]
            la[:, cb + 16 : cb + 24] = qc_t[:, rs]
        la = la.astype(BF16).reshape(4, 128, 96)          # (kc, kp, col)
        lv_i = np.ascontiguousarray(
            np.concatenate([la, v2h], axis=2)             # (kc, kp, 96+768)
            .transpose(1, 0, 2)
        ).reshape(128, 4 * 864)
        in_maps.append(
            {
                "lv": lv_i,
                "ut4": ut4_i,
                "cst": cst_dev,
            }
        )
    return in_maps, fold_const


def _run(inputs, trace=False, trace_kwargs=None):
    from concourse.bass_utils import run_bass_kernel_spmd

    coords = np.asarray(inputs["point_trajs_gt_coord"], dtype=np.float32)
    mask = np.asarray(inputs["point_trajs_visibility_mask"], dtype=np.float32)
    pos = np.asarray(inputs["pos_embed"], dtype=np.float32)
    w1 = np.asarray(inputs["fc1_w"], dtype=np.float32)
    b1 = np.asarray(inputs["fc1_b"], dtype=np.float32)
    w2 = np.asarray(inputs["fc_out_w"], dtype=np.float32)
    b2 = np.asarray(inputs["fc_out_b"], dtype=np.float32)

    in_maps, fold_const = _prep_inputs(coords, mask, pos, w1, b1, w2, b2)
    if fold_const not in _CACHED_NC:
        _CACHED_NC[fold_const] = _build_nc(fold_const)
    nc = _CACHED_NC[fold_const]

    res = run_bass_kernel_spmd(
        nc, in_maps, list(range(N_CORES)), trace=trace, **(trace_kwargs or {})
    )
    shards = [res.results[i]["out"] for i in range(N_CORES)]
    full = np.concatenate(shards, axis=0).reshape(B, T, M, D_OUT)
    return full.astype(np.float32), res


def kernel(**inputs):
    out, _ = _run(inputs, trace=False)
    return out


# revision 13
# speedup vs baseline: 1.0130x; 1.0130x over previous
"""Trainium2 Bass kernel for nn_CrossmotionModule (gnn_message_passing).

Reference computation (B=4, M=256, T=64, Dm=512, E=768):
    rel[b,m,t,n,k] = (c[b,m,t,k] - c[b,n,t,k]) * vis[b,m,t] * vis[b,n,t]
    fea[b,t,m,(n,k)] = rel                  # (B,T,M,512)
    h   = fea @ W1 + b1                     # (B,T,M,512)
    out = [h, pos] @ W2 + b2                # (B,T,M,768)

Algebraic collapse: with p = vis (B,T,M), u0 = p*c0, u1 = p*c1, the output is
a rank-3 outer product per (b,t) plus a constant:
    out[bt,m,e] = u0[m]*G0[e] + u1[m]*G1[e] - p[m]*G2[e] + const[m,e]
where, with the host-folded fused weight V2 = W1 @ W2[:512] (512, 768):
    G0[e] = sum_n p[n] V2[2n, e]
    G1[e] = sum_n p[n] V2[2n+1, e]
    G2[e] = sum_nk (p*c)[nk] V2[nk, e]
    const = b1 @ W2[:512] + pos @ W2[512:] + b2

Single-pass bf16 pipeline (tolerance is rel_l2 < 2e-2; this lands ~3e-3):
bf16 in / f32 PSUM accumulate / bf16 out, host upcasts to float32. When
const is m-independent (pos_embed == 0, always true here) it is folded into
the matmul as a 4th contraction row (lhsT ones, rhs const_e), so the PSUM
drain is a pure cast-copy, split per 512-col PSUM bank between the DVE and
ACT engines. Input/bounce DMAs dispatch from gpsimd (cheap parallel queue);
output DMAs alternate sync/gpsimd.

Sharding: data-parallel over bt = (b,t) flattened; 256 rows / 8 cores = 32
rows per core. Weights replicated. No cross-device communication.
"""

import ml_dtypes
import numpy as np

B, M, T = 4, 256, 64
D_MOT, D_ABS, D_OUT = 512, 512, 768
N_CORES = 8
BT = B * T            # 256
R = BT // N_CORES     # 32 bt rows per core
E = D_OUT

BF16 = ml_dtypes.bfloat16

_CACHED_NC = {}


def _build_nc(fold_const):
    """Build the SPMD Bass program (identical for all 8 cores)."""
    import concourse.bacc as bacc
    import concourse.bass as bass
    import concourse.mybir as mybir
    import concourse.tile as tile

    f32 = mybir.dt.float32
    bf16 = mybir.dt.bfloat16
    PSUM = bass.MemorySpace.PSUM

    nc = bacc.Bacc("TRN2", target_bir_lowering=False, debug=False)

    # Per-core inputs (host-prepared layouts; see _prep_inputs).
    # lv: per k-chunk [la_k (96) | vh_k (768)] so one DMA unblocks matmul k.
    lv_d = nc.dram_tensor("lv", [128, 4 * 864], bf16, kind="ExternalInput")
    ut4_d = nc.dram_tensor("ut4", [4, R * 256], bf16, kind="ExternalInput")
    if fold_const:
        cst_d = nc.dram_tensor("cst", [1, R * E], bf16, kind="ExternalInput")
    else:
        cst_d = nc.dram_tensor("cst", [128, 1536], f32, kind="ExternalInput")
    out_d = nc.dram_tensor("out", [R, M, E], bf16, kind="ExternalOutput")
    # DRAM bounce for the G reshuffle. Row (ck*3 + j) holds chunk ck of Gj:
    # 8 consecutive r values x 768 e. Written as a (j r8) partition scatter
    # per chunk, read back as 3 fat rows per chunk.
    gscr_d = nc.dram_tensor("gscr", [12, 8 * E], bf16)

    CK = (R // 4) * E  # per-chunk column count of g4's (r e) axis

    with tile.TileContext(nc) as tc:
        with tc.tile_pool(name="persist", bufs=1) as pers:
            ut4_sb = pers.tile([4, R * 256], bf16)
            g4_sb = pers.tile([4, R * E], bf16)
            scr_sb = pers.tile([1, 8], f32)
            if not fold_const:
                cst_sb = pers.tile([128, 1536], f32)

            # ---- prologue: G[(ck,j,r8), e] via the fused weight V2 ----
            with (
                tc.tile_pool(name="pro", bufs=1) as pro,
                tc.tile_pool(name="prop", bufs=1, space=PSUM) as prop,
            ):
                lv_sb = pro.tile([128, 4 * 864], bf16)
                g96_sb = pro.tile([96, E], bf16)
                # lv chunks + bounce writes on the gpsimd (SWDGE) queue in
                # that order; ut4/cst (not needed until the main loop) on the
                # ACT HWDGE queue; bounce reads + outputs on SP.
                for kc in range(4):
                    nc.gpsimd.dma_start(
                        lv_sb[:, kc * 864 : (kc + 1) * 864],
                        lv_d[:, kc * 864 : (kc + 1) * 864],
                    )
                nc.scalar.dma_start(ut4_sb[:], ut4_d[:])
                if fold_const:
                    for ck in range(4):
                        nc.scalar.dma_start(
                            g4_sb[3:4, ck * CK : (ck + 1) * CK],
                            cst_d[:, ck * CK : (ck + 1) * CK],
                        )
                else:
                    nc.scalar.dma_start(cst_sb[:], cst_d[:])
                # Warm the ACT activation table (after ACT's dma dispatches).
                nc.gpsimd.memset(scr_sb[:], 0.0)
                nc.scalar.copy(scr_sb[0:1, 4:8], scr_sb[0:1, 0:4])

                g_ps = prop.tile([96, E], f32)
                for kc in range(4):
                    for lo, hi in ((0, 512), (512, 768)):
                        nc.tensor.matmul(
                            g_ps[:, lo:hi],
                            lv_sb[:, kc * 864 : kc * 864 + 96],
                            lv_sb[:, kc * 864 + 96 + lo : kc * 864 + 96 + hi],
                            start=(kc == 0),
                            stop=(kc == 3),
                        )
                # PE-rate probes in the prologue shadow: which of K / output
                # partition count gates the streaming rate? (read via trace)
                probe = prop.tile([128, 512], f32)
                nc.tensor.matmul(probe[0:128, :], ut4_sb[0:4, 0:128], ut4_sb[0:4, 0:512])
                nc.tensor.matmul(probe[0:96, :], ut4_sb[0:4, 0:96], ut4_sb[0:4, 0:512])
                nc.tensor.matmul(probe[0:64, :], ut4_sb[0:4, 0:64], ut4_sb[0:4, 0:512])
                nc.tensor.matmul(probe[0:128, :], lv_sb[0:16, 0:128], lv_sb[0:16, 0:512])
                nc.tensor.matmul(probe[0:128, :], lv_sb[0:32, 0:128], lv_sb[0:32, 0:512])
                nc.tensor.matmul(probe[0:128, :], lv_sb[0:128, 0:128], lv_sb[0:128, 0:512])

                # Cast all of G (DVE partition starts must be 32-aligned),
                # then per chunk: scatter-write, fat read-back.
                nc.vector.tensor_copy(g96_sb[:], g_ps[:])
                for ck in range(4):
                    p0 = ck * 24
                    nc.gpsimd.dma_start(
                        gscr_d[3 * ck : 3 * ck + 3].rearrange(
                            "j (r e) -> (j r) e", r=8
                        ),
                        g96_sb[p0 : p0 + 24, :],
                    )
                    nc.sync.dma_start(
                        g4_sb[0:3, ck * CK : (ck + 1) * CK],
                        gscr_d[3 * ck : 3 * ck + 3],
                    )

            # ---- main loop: out[r, m, e] = U4_r^T G4_r (+ const) ----
            # m = 2p + w: partition p holds (w=0, w=1) halves side by side so
            # each DMA line is 3KB contiguous in DRAM. PSUM tiles are single
            # banks (512 f32) so drains hand banks back at fine grain.
            with (
                tc.tile_pool(name="mp", bufs=8, space=PSUM) as mp,
                tc.tile_pool(name="op", bufs=8) as op,
            ):
                for r in range(R):
                    u_w0 = ut4_sb[:, r * 256 : r * 256 + 128]
                    u_w1 = ut4_sb[:, r * 256 + 128 : r * 256 + 256]
                    g0 = r * E
                    ps0 = mp.tile([128, 512], f32, tag="ps")
                    ps1 = mp.tile([128, 512], f32, tag="ps")
                    ps2 = mp.tile([128, 512], f32, tag="ps")
                    nc.tensor.matmul(ps0[:], u_w0, g4_sb[:, g0 : g0 + 512])
                    nc.tensor.matmul(ps1[:, 0:256], u_w0, g4_sb[:, g0 + 512 : g0 + 768])
                    nc.tensor.matmul(ps1[:, 256:512], u_w1, g4_sb[:, g0 : g0 + 256])
                    nc.tensor.matmul(ps2[:], u_w1, g4_sb[:, g0 + 256 : g0 + 768])
                    out_sb = op.tile([128, 1536], bf16, tag="out_sb")
                    if fold_const:
                        # Alternate engines across banks and rows for balance.
                        drains = (
                            (nc.vector.tensor_copy, nc.scalar.copy, nc.vector.tensor_copy)
                            if r % 2 == 0
                            else (nc.scalar.copy, nc.vector.tensor_copy, nc.scalar.copy)
                        )
                        drains[0](out_sb[:, 0:512], ps0[:])
                        drains[1](out_sb[:, 512:1024], ps1[:])
                        drains[2](out_sb[:, 1024:1536], ps2[:])
                    else:
                        nc.vector.tensor_add(out_sb[:, 0:512], ps0[:], cst_sb[:, 0:512])
                        nc.vector.tensor_add(out_sb[:, 512:1024], ps1[:], cst_sb[:, 512:1024])
                        nc.vector.tensor_add(out_sb[:, 1024:1536], ps2[:], cst_sb[:, 1024:1536])
                    dst = out_d[r].rearrange("(p w) e -> p (w e)", w=2)
                    if r == R - 1:
                        # Split the last row so the tail transfer is short.
                        nc.sync.dma_start(dst[:, 0:768], out_sb[:, 0:768])
                        nc.sync.dma_start(dst[:, 768:1536], out_sb[:, 768:1536])
                    else:
                        nc.sync.dma_start(dst, out_sb[:])
    nc.compile()
    return nc


def _prep_inputs(coords, mask, pos, w1, b1, w2, b2):
    """Host-side input sharding + weight-only constant folding."""
    nan0 = np.isnan(coords[..., 0])
    c = np.nan_to_num(coords)
    vis = np.where(nan0, np.float32(0.0), mask).astype(np.float32)

    p_all = np.ascontiguousarray(vis.transpose(0, 2, 1)).reshape(BT, M)
    c_bt = np.ascontiguousarray(c.transpose(0, 2, 1, 3)).reshape(BT, M, 2)
    q_all = (p_all[:, :, None] * c_bt).reshape(BT, 2 * M).astype(np.float32)

    W2t = w2[:D_MOT]
    W2b = w2[D_MOT:]
    const = (b1 @ W2t + b2)[None, :] + pos @ W2b          # (M, 768)
    fold_const = bool(np.all(const == const[0:1]))
    if fold_const:
        cst_dev = np.tile(const[0].astype(BF16)[None, :], (1, R)).reshape(1, R * E)
    else:
        cst_dev = np.ascontiguousarray(
            const.astype(np.float32).reshape(128, 2, D_OUT)
        ).reshape(128, 1536)

    # Fused weight V2 = W1 @ W2t in bf16, 128 contraction rows per chunk.
    v2h = (w1 @ W2t).astype(np.float32).astype(BF16)      # (512, 768)
    v2h = v2h.reshape(4, 128, D_OUT)                      # (kc, kp, e)

    # U4 rows pair with G4 rows [G0; G1; G2; const]: [u0; u1; -p; ones].
    u0 = q_all[:, 0::2]
    u1 = q_all[:, 1::2]
    ones = np.ones_like(p_all)
    U4 = np.stack([u0, u1, -p_all, ones], axis=0)         # (4, BT, M)
    U4 = U4.reshape(4, BT, 128, 2).transpose(0, 1, 3, 2)  # m = 2p+w
    U4 = U4.astype(BF16)                                  # (4, BT, 2, 128)

    in_maps = []
    for i in range(N_CORES):
        rows = slice(i * R, (i + 1) * R)
        ut4_i = np.ascontiguousarray(U4[:, rows]).reshape(4, R * 256)

        # L columns ordered (ck, j, r8) so bounce chunks are partition-
        # contiguous: col ck*24 + j*8 + r8 <-> G row j of bt row ck*8+r8.
        # L rows (the 512 contraction dim): j=0 picks even k (G0), j=1 odd
        # (G1), j=2 is Q (G2).
        pc_t = p_all[rows].T                              # (256, R)
        qc_t = q_all[rows].T                              # (512, R)
        la = np.zeros((512, 96), np.float32)
        for ck in range(4):
            rs = slice(ck * 8, ck * 8 + 8)
            cb = ck * 24
            la[0::2, cb : cb + 8] = pc_t[:, rs]
            la[1::2, cb + 8 : cb + 16] = pc_t[:, rs]
            la[:, cb + 16 : cb + 24] = qc_t[:, rs]
        la = la.astype(BF16).reshape(4, 128, 96)          # (kc, kp, col)
        lv_i = np.ascontiguousarray(
            np.concatenate([la, v2h], axis=2)             # (kc, kp, 96+768)
            .transpose(1, 0, 2)
        ).reshape(128, 4 * 864)
        in_maps.append(
            {
                "lv": lv_i,
                "ut4": ut4_i,
                "cst": cst_dev,
            }
        )
    return in_maps, fold_const


def _run(inputs, trace=False, trace_kwargs=None):
    from concourse.bass_utils import run_bass_kernel_spmd

    coords = np.asarray(inputs["point_trajs_gt_coord"], dtype=np.float32)
    mask = np.asarray(inputs["point_trajs_visibility_mask"], dtype=np.float32)
    pos = np.asarray(inputs["pos_embed"], dtype=np.float32)
    w1 = np.asarray(inputs["fc1_w"], dtype=np.float32)
    b1 = np.asarray(inputs["fc1_b"], dtype=np.float32)
    w2 = np.asarray(inputs["fc_out_w"], dtype=np.float32)
    b2 = np.asarray(inputs["fc_out_b"], dtype=np.float32)

    in_maps, fold_const = _prep_inputs(coords, mask, pos, w1, b1, w2, b2)
    if fold_const not in _CACHED_NC:
        _CACHED_NC[fold_const] = _build_nc(fold_const)
    nc = _CACHED_NC[fold_const]

    res = run_bass_kernel_spmd(
        nc, in_maps, list(range(N_CORES)), trace=trace, **(trace_kwargs or {})
    )
    shards = [res.results[i]["out"] for i in range(N_CORES)]
    full = np.concatenate(shards, axis=0).reshape(B, T, M, D_OUT)
    return full.astype(np.float32), res


def kernel(**inputs):
    out, _ = _run(inputs, trace=False)
    return out


# revision 16
# speedup vs baseline: 1.0293x; 1.0161x over previous
"""Trainium2 Bass kernel for nn_CrossmotionModule (gnn_message_passing).

Reference computation (B=4, M=256, T=64, Dm=512, E=768):
    rel[b,m,t,n,k] = (c[b,m,t,k] - c[b,n,t,k]) * vis[b,m,t] * vis[b,n,t]
    fea[b,t,m,(n,k)] = rel                  # (B,T,M,512)
    h   = fea @ W1 + b1                     # (B,T,M,512)
    out = [h, pos] @ W2 + b2                # (B,T,M,768)

Algebraic collapse: with p = vis (B,T,M), u0 = p*c0, u1 = p*c1, the output is
a rank-3 outer product per (b,t) plus a constant:
    out[bt,m,e] = u0[m]*G0[e] + u1[m]*G1[e] - p[m]*G2[e] + const[m,e]
where, with the host-folded fused weight V2 = W1 @ W2[:512] (512, 768):
    G0[e] = sum_n p[n] V2[2n, e]
    G1[e] = sum_n p[n] V2[2n+1, e]
    G2[e] = sum_nk (p*c)[nk] V2[nk, e]
    const = b1 @ W2[:512] + pos @ W2[512:] + b2

Single-pass bf16 pipeline (tolerance is rel_l2 < 2e-2; this lands ~3e-3):
bf16 in / f32 PSUM accumulate / bf16 out, host upcasts to float32. When
const is m-independent (pos_embed == 0, always true here) it is folded into
the matmul as a 4th contraction row (lhsT ones, rhs const_e), so the PSUM
drain is a pure cast-copy, split per 512-col PSUM bank between the DVE and
ACT engines. Input/bounce DMAs dispatch from gpsimd (cheap parallel queue);
output DMAs alternate sync/gpsimd.

Sharding: data-parallel over bt = (b,t) flattened; 256 rows / 8 cores = 32
rows per core. Weights replicated. No cross-device communication.
"""

import ml_dtypes
import numpy as np

B, M, T = 4, 256, 64
D_MOT, D_ABS, D_OUT = 512, 512, 768
N_CORES = 8
BT = B * T            # 256
R = BT // N_CORES     # 32 bt rows per core
E = D_OUT

BF16 = ml_dtypes.bfloat16

_CACHED_NC = {}


def _build_nc(fold_const):
    """Build the SPMD Bass program (identical for all 8 cores)."""
    import concourse.bacc as bacc
    import concourse.bass as bass
    import concourse.mybir as mybir
    import concourse.tile as tile

    f32 = mybir.dt.float32
    bf16 = mybir.dt.bfloat16
    PSUM = bass.MemorySpace.PSUM

    nc = bacc.Bacc("TRN2", target_bir_lowering=False, debug=False)

    # Per-core inputs (host-prepared layouts; see _prep_inputs).
    # lv: per k-chunk [la_k (96) | vh_k (768)] so one DMA unblocks matmul k.
    lv_d = nc.dram_tensor("lv", [128, 4 * 864], bf16, kind="ExternalInput")
    ut4_d = nc.dram_tensor("ut4", [4, R * 256], bf16, kind="ExternalInput")
    if fold_const:
        cst_d = nc.dram_tensor("cst", [1, R * E], bf16, kind="ExternalInput")
    else:
        cst_d = nc.dram_tensor("cst", [128, 1536], f32, kind="ExternalInput")
    out_d = nc.dram_tensor("out", [R, M, E], bf16, kind="ExternalOutput")
    # DRAM bounce for the G reshuffle. Row (ck*3 + j) holds chunk ck of Gj:
    # 8 consecutive r values x 768 e. Written as a (j r8) partition scatter
    # per chunk, read back as 3 fat rows per chunk.
    gscr_d = nc.dram_tensor("gscr", [12, 8 * E], bf16)

    CK = (R // 4) * E  # per-chunk column count of g4's (r e) axis

    with tile.TileContext(nc) as tc:
        with tc.tile_pool(name="persist", bufs=1) as pers:
            ut4_sb = pers.tile([4, R * 256], bf16)
            g4_sb = pers.tile([4, R * E], bf16)
            scr_sb = pers.tile([1, 8], f32)
            if not fold_const:
                cst_sb = pers.tile([128, 1536], f32)

            # ---- prologue: G[(ck,j,r8), e] via the fused weight V2 ----
            with (
                tc.tile_pool(name="pro", bufs=1) as pro,
                tc.tile_pool(name="prop", bufs=1, space=PSUM) as prop,
            ):
                lv_sb = pro.tile([128, 4 * 864], bf16)
                g96_sb = pro.tile([96, E], bf16)
                # lv chunks + bounce write/read on the SP HWDGE queue (565ns
                # dispatches; gpsimd SWDGE gen measures ~2us per DMA);
                # ut4/cst (not needed until the main loop) on the ACT queue.
                for kc in range(4):
                    nc.sync.dma_start(
                        lv_sb[:, kc * 864 : (kc + 1) * 864],
                        lv_d[:, kc * 864 : (kc + 1) * 864],
                    )
                nc.scalar.dma_start(ut4_sb[:], ut4_d[:])
                if fold_const:
                    for ck in range(4):
                        nc.scalar.dma_start(
                            g4_sb[3:4, ck * CK : (ck + 1) * CK],
                            cst_d[:, ck * CK : (ck + 1) * CK],
                        )
                else:
                    nc.scalar.dma_start(cst_sb[:], cst_d[:])
                # Warm the ACT activation table (after ACT's dma dispatches).
                nc.gpsimd.memset(scr_sb[:], 0.0)
                nc.scalar.copy(scr_sb[0:1, 4:8], scr_sb[0:1, 0:4])

                g_ps = prop.tile([96, E], f32)
                for kc in range(4):
                    for lo, hi in ((0, 512), (512, 768)):
                        nc.tensor.matmul(
                            g_ps[:, lo:hi],
                            lv_sb[:, kc * 864 : kc * 864 + 96],
                            lv_sb[:, kc * 864 + 96 + lo : kc * 864 + 96 + hi],
                            start=(kc == 0),
                            stop=(kc == 3),
                        )
                # Cast all of G (DVE partition starts must be 32-aligned),
                # then per chunk: scatter-write, fat read-back.
                nc.vector.tensor_copy(g96_sb[:], g_ps[:])
                for ck in range(4):
                    p0 = ck * 24
                    nc.sync.dma_start(
                        gscr_d[3 * ck : 3 * ck + 3].rearrange(
                            "j (r e) -> (j r) e", r=8
                        ),
                        g96_sb[p0 : p0 + 24, :],
                    )
                    nc.sync.dma_start(
                        g4_sb[0:3, ck * CK : (ck + 1) * CK],
                        gscr_d[3 * ck : 3 * ck + 3],
                    )

            # ---- main loop: out[r, m, e] = U4_r^T G4_r (+ const) ----
            # m = 2p + w: partition p holds (w=0, w=1) halves side by side so
            # each DMA line is 3KB contiguous in DRAM. PSUM tiles are single
            # banks (512 f32) so drains hand banks back at fine grain.
            with (
                tc.tile_pool(name="mp", bufs=8, space=PSUM) as mp,
                tc.tile_pool(name="op", bufs=8) as op,
            ):
                for r in range(R):
                    u_w0 = ut4_sb[:, r * 256 : r * 256 + 128]
                    u_w1 = ut4_sb[:, r * 256 + 128 : r * 256 + 256]
                    g0 = r * E
                    ps0 = mp.tile([128, 512], f32, tag="ps")
                    ps1 = mp.tile([128, 512], f32, tag="ps")
                    ps2 = mp.tile([128, 512], f32, tag="ps")
                    nc.tensor.matmul(ps0[:], u_w0, g4_sb[:, g0 : g0 + 512])
                    nc.tensor.matmul(ps1[:, 0:256], u_w0, g4_sb[:, g0 + 512 : g0 + 768])
                    nc.tensor.matmul(ps1[:, 256:512], u_w1, g4_sb[:, g0 : g0 + 256])
                    nc.tensor.matmul(ps2[:], u_w1, g4_sb[:, g0 + 256 : g0 + 768])
                    out_sb = op.tile([128, 1536], bf16, tag="out_sb")
                    if fold_const:
                        # Alternate engines across banks and rows for balance.
                        drains = (
                            (nc.vector.tensor_copy, nc.scalar.copy, nc.vector.tensor_copy)
                            if r % 2 == 0
                            else (nc.scalar.copy, nc.vector.tensor_copy, nc.scalar.copy)
                        )
                        drains[0](out_sb[:, 0:512], ps0[:])
                        drains[1](out_sb[:, 512:1024], ps1[:])
                        drains[2](out_sb[:, 1024:1536], ps2[:])
                    else:
                        nc.vector.tensor_add(out_sb[:, 0:512], ps0[:], cst_sb[:, 0:512])
                        nc.vector.tensor_add(out_sb[:, 512:1024], ps1[:], cst_sb[:, 512:1024])
                        nc.vector.tensor_add(out_sb[:, 1024:1536], ps2[:], cst_sb[:, 1024:1536])
                    dst = out_d[r].rearrange("(p w) e -> p (w e)", w=2)
                    if r == R - 1:
                        # Split the last row so the tail transfer is short.
                        nc.sync.dma_start(dst[:, 0:768], out_sb[:, 0:768])
                        nc.sync.dma_start(dst[:, 768:1536], out_sb[:, 768:1536])
                    else:
                        nc.sync.dma_start(dst, out_sb[:])
    nc.compile()
    return nc


def _prep_inputs(coords, mask, pos, w1, b1, w2, b2):
    """Host-side input sharding + weight-only constant folding."""
    nan0 = np.isnan(coords[..., 0])
    c = np.nan_to_num(coords)
    vis = np.where(nan0, np.float32(0.0), mask).astype(np.float32)

    p_all = np.ascontiguousarray(vis.transpose(0, 2, 1)).reshape(BT, M)
    c_bt = np.ascontiguousarray(c.transpose(0, 2, 1, 3)).reshape(BT, M, 2)
    q_all = (p_all[:, :, None] * c_bt).reshape(BT, 2 * M).astype(np.float32)

    W2t = w2[:D_MOT]
    W2b = w2[D_MOT:]
    const = (b1 @ W2t + b2)[None, :] + pos @ W2b          # (M, 768)
    fold_const = bool(np.all(const == const[0:1]))
    if fold_const:
        cst_dev = np.tile(const[0].astype(BF16)[None, :], (1, R)).reshape(1, R * E)
    else:
        cst_dev = np.ascontiguousarray(
            const.astype(np.float32).reshape(128, 2, D_OUT)
        ).reshape(128, 1536)

    # Fused weight V2 = W1 @ W2t in bf16, 128 contraction rows per chunk.
    v2h = (w1 @ W2t).astype(np.float32).astype(BF16)      # (512, 768)
    v2h = v2h.reshape(4, 128, D_OUT)                      # (kc, kp, e)

    # U4 rows pair with G4 rows [G0; G1; G2; const]: [u0; u1; -p; ones].
    u0 = q_all[:, 0::2]
    u1 = q_all[:, 1::2]
    ones = np.ones_like(p_all)
    U4 = np.stack([u0, u1, -p_all, ones], axis=0)         # (4, BT, M)
    U4 = U4.reshape(4, BT, 128, 2).transpose(0, 1, 3, 2)  # m = 2p+w
    U4 = U4.astype(BF16)                                  # (4, BT, 2, 128)

    in_maps = []
    for i in range(N_CORES):
        rows = slice(i * R, (i + 1) * R)
        ut4_i = np.ascontiguousarray(U4[:, rows]).reshape(4, R * 256)

        # L columns ordered (ck, j, r8) so bounce chunks are partition-
        # contiguous: col ck*24 + j*8 + r8 <-> G row j of bt row ck*8+r8.
        # L rows (the 512 contraction dim): j=0 picks even k (G0), j=1 odd
        # (G1), j=2 is Q (G2).
        pc_t = p_all[rows].T                              # (256, R)
        qc_t = q_all[rows].T                              # (512, R)
        la = np.zeros((512, 96), np.float32)
        for ck in range(4):
            rs = slice(ck * 8, ck * 8 + 8)
            cb = ck * 24
            la[0::2, cb : cb + 8] = pc_t[:, rs]
            la[1::2, cb + 8 : cb + 16] = pc_t[:, rs]
            la[:, cb + 16 : cb + 24] = qc_t[:, rs]
        la = la.astype(BF16).reshape(4, 128, 96)          # (kc, kp, col)
        lv_i = np.ascontiguousarray(
            np.concatenate([la, v2h], axis=2)             # (kc, kp, 96+768)
            .transpose(1, 0, 2)
        ).reshape(128, 4 * 864)
        in_maps.append(
            {
                "lv": lv_i,
                "ut4": ut4_i,
                "cst": cst_dev,
            }
        )
    return in_maps, fold_const


def _run(inputs, trace=False, trace_kwargs=None):
    from concourse.bass_utils import run_bass_kernel_spmd

    coords = np.asarray(inputs["point_trajs_gt_coord"], dtype=np.float32)
    mask = np.asarray(inputs["point_trajs_visibility_mask"], dtype=np.float32)
    pos = np.asarray(inputs["pos_embed"], dtype=np.float32)
    w1 = np.asarray(inputs["fc1_w"], dtype=np.float32)
    b1 = np.asarray(inputs["fc1_b"], dtype=np.float32)
    w2 = np.asarray(inputs["fc_out_w"], dtype=np.float32)
    b2 = np.asarray(inputs["fc_out_b"], dtype=np.float32)

    in_maps, fold_const = _prep_inputs(coords, mask, pos, w1, b1, w2, b2)
    if fold_const not in _CACHED_NC:
        _CACHED_NC[fold_const] = _build_nc(fold_const)
    nc = _CACHED_NC[fold_const]

    res = run_bass_kernel_spmd(
        nc, in_maps, list(range(N_CORES)), trace=trace, **(trace_kwargs or {})
    )
    shards = [res.results[i]["out"] for i in range(N_CORES)]
    full = np.concatenate(shards, axis=0).reshape(B, T, M, D_OUT)
    return full.astype(np.float32), res


def kernel(**inputs):
    out, _ = _run(inputs, trace=False)
    return out
